# revision 4
# baseline (speedup 1.0000x reference)
"""Grouped GEMM (MoE expert layers) on 8 Trainium2 NeuronCores — v4.

v3 (639.6 us) = 614 us dense matmuls + 17.9 us startup + ~8 us tail.
v4 attacks the edges:
  - ~32 warmup matmuls on a zeroed tile run during the initial DMA wait,
    so the PE HAM clock-gate is already at 2.4 GHz when real matmuls
    start (v3 paid ~4 us of 1.2 GHz cold matmuls).
  - expert 0's first transfers are sliced finer (2-kt w groups, 512-token
    first x chunk) so the first real matmul starts ~5 us earlier.
  - the last token tile runs all ps0 matmuls before ps1's, letting the
    final CAST + output DMA of the first half overlap the second half.
"""
import os
import numpy as np
import ml_dtypes

E, IN, OUT, T, NCORES = 8, 2048, 5632, 16384, 8
OUT_C = OUT // NCORES          # 704 out-features per core
P = 128                        # partitions
KT = IN // P                   # 16 k-tiles of 128
NSPLIT = 352                   # psum bank-sized halves of OUT_C
WARMUP_MM = int(os.environ.get("V4_WARMUP", "32"))


def _pad_segments(offsets):
    sizes = np.diff(offsets).astype(int)
    padded = [(-(-s // P)) * P for s in sizes]
    return list(sizes), padded, int(sum(padded))


def _build_program(padded_sizes, dt_in, dt_out):
    import concourse.bass as bass
    import concourse.mybir as mybir
    from concourse.tile import TileContext
    from wait_legalize_embed import legalize_waits

    Tp = sum(padded_sizes)
    nc = bass.Bass()
    xT_d = nc.dram_tensor("xT", [IN, Tp], dt_in, kind="ExternalInput")
    wT_d = nc.dram_tensor("wT", [E, IN, OUT_C], dt_in, kind="ExternalInput")
    out_d = nc.dram_tensor("out", [Tp, OUT_C], dt_out, kind="ExternalOutput")

    xT_r = xT_d.rearrange("(kt p) t -> p kt t", p=P)

    # expert-order list of (global_tile0, ntiles) segments
    segs = []
    tb = 0
    for e in range(E):
        nt = padded_sizes[e] // P
        if nt:
            segs.append((e, tb, nt))
        tb += nt
    last_seg = len(segs) - 1

    with TileContext(nc) as tc:
        with tc.tile_pool(name="wpool", bufs=2) as wpool, \
             tc.tile_pool(name="xpool", bufs=2) as xpool, \
             tc.tile_pool(name="zpool", bufs=1) as zpool, \
             tc.tile_pool(name="opool", bufs=6) as opool, \
             tc.tile_pool(name="ppool", bufs=8, space="PSUM") as ppool:
            # ---- PE warmup: run garbage matmuls while the first DMAs land
            # (the psum comes from the shared "ps" ring, so it is recycled
            # once the first chunk needs all 8 banks)
            if WARMUP_MM:
                z_sb = zpool.tile([P, P], dt_in, tag="z", name="zsb")
                nc.vector.memset(z_sb[:], 0.0)
                psw = ppool.tile([P, NSPLIT], mybir.dt.float32, tag="ps",
                                 name="psw")
                for _ in range(WARMUP_MM):
                    nc.tensor.matmul(psw[:, 0:P], z_sb[:], z_sb[:],
                                     start=True, stop=True)

            for si, (e, tile0, ntiles) in enumerate(segs):
                first = si == 0
                # w DMA: 2-kt groups for the first expert, 4-kt after
                kgn_w = 2 if first else 4
                ngw = KT // kgn_w
                w_r = wT_d[e].rearrange("(kt p) n -> p kt n", p=P)
                wtag = "v" if first else "w"
                w_sb = [wpool.tile([P, kgn_w, OUT_C], dt_in,
                                   tag=f"{wtag}{g}", name=f"wsb{g}",
                                   bufs=1 if first else 2)
                        for g in range(ngw)]
                for g in range(ngw):
                    nc.scalar.dma_start(
                        out=w_sb[g][:],
                        in_=w_r[:, g * kgn_w : (g + 1) * kgn_w, :],
                    )

                # token chunking: first chunk of first expert is 4 tiles
                chunks = []
                t = 0
                while t < ntiles:
                    step = 4 if (first and t == 0) else 8
                    cur = min(step, ntiles - t)
                    chunks.append((t, cur))
                    t += cur
                for ci, (tt0, cur) in enumerate(chunks):
                    t0 = (tile0 + tt0) * P
                    small = first and ci == 0
                    kgn_x = 2 if small else 4
                    ngx = KT // kgn_x
                    xtag = "y" if small else "x"
                    xw = 4 * P if small else 8 * P
                    x_sb = [xpool.tile([P, kgn_x, xw], dt_in,
                                       tag=f"{xtag}{g}", name=f"xsb{g}",
                                       bufs=1 if small else 2)
                            for g in range(ngx)]
                    for g in range(ngx):
                        nc.sync.dma_start(
                            out=x_sb[g][:, :, : cur * P],
                            in_=xT_r[:, g * kgn_x : (g + 1) * kgn_x,
                                     t0 : t0 + cur * P],
                        )
                    if small:
                        # kt-group-major over the whole first chunk: each
                        # newly-landed 2-kt slice immediately feeds 4x4
                        # matmuls, so the PE never outruns the DMA cadence.
                        # Uses 2*cur psum tiles (= all 8 banks at cur=4).
                        pss = [
                            (ppool.tile([P, NSPLIT], mybir.dt.float32,
                                        tag="ps", name=f"psA{j}"),
                             ppool.tile([P, NSPLIT], mybir.dt.float32,
                                        tag="ps", name=f"psB{j}"))
                            for j in range(cur)
                        ]
                        for kt in range(KT):
                            gx, kx = kt // kgn_x, kt % kgn_x
                            gw, kw = kt // kgn_w, kt % kgn_w
                            for j in range(cur):
                                lhsT = x_sb[gx][:, kx, j * P : (j + 1) * P]
                                nc.tensor.matmul(
                                    pss[j][0][:], lhsT,
                                    w_sb[gw][:, kw, 0:NSPLIT],
                                    start=(kt == 0), stop=(kt == KT - 1),
                                )
                                nc.tensor.matmul(
                                    pss[j][1][:], lhsT,
                                    w_sb[gw][:, kw, NSPLIT:OUT_C],
                                    start=(kt == 0), stop=(kt == KT - 1),
                                )
                        for j in range(cur):
                            o_sb = opool.tile([P, OUT_C], dt_out, tag="o")
                            nc.vector.tensor_copy(o_sb[:, 0:NSPLIT], pss[j][0][:])
                            nc.vector.tensor_copy(o_sb[:, NSPLIT:OUT_C], pss[j][1][:])
                            row = t0 + j * P
                            nc.scalar.dma_start(
                                out=out_d[row : row + P, :], in_=o_sb[:]
                            )
                        continue
                    for j in range(cur):
                        ps0 = ppool.tile([P, NSPLIT], mybir.dt.float32, tag="ps")
                        ps1 = ppool.tile([P, NSPLIT], mybir.dt.float32, tag="ps")
                        lhs = []
                        for kt in range(KT):
                            gx, kx = kt // kgn_x, kt % kgn_x
                            lhs.append(x_sb[gx][:, kx, j * P : (j + 1) * P])
                        rhs = []
                        for kt in range(KT):
                            gw, kw = kt // kgn_w, kt % kgn_w
                            rhs.append(w_sb[gw][:, kw, :])
                        tail = si == last_seg and ci == len(chunks) - 1 \
                            and j == cur - 1
                        o_sb = opool.tile([P, OUT_C], dt_out, tag="o")
                        row = t0 + j * P
                        if not tail:
                            for kt in range(KT):
                                nc.tensor.matmul(
                                    ps0[:], lhs[kt], rhs[kt][:, 0:NSPLIT],
                                    start=(kt == 0), stop=(kt == KT - 1),
                                )
                                nc.tensor.matmul(
                                    ps1[:], lhs[kt], rhs[kt][:, NSPLIT:OUT_C],
                                    start=(kt == 0), stop=(kt == KT - 1),
                                )
                            nc.vector.tensor_copy(o_sb[:, 0:NSPLIT], ps0[:])
                            nc.vector.tensor_copy(o_sb[:, NSPLIT:OUT_C], ps1[:])
                            nc.scalar.dma_start(
                                out=out_d[row : row + P, :], in_=o_sb[:]
                            )
                        else:
                            # last tile: finish ps0 early so its cast +
                            # output DMA overlap ps1's matmuls
                            for kt in range(KT):
                                nc.tensor.matmul(
                                    ps0[:], lhs[kt], rhs[kt][:, 0:NSPLIT],
                                    start=(kt == 0), stop=(kt == KT - 1),
                                )
                            nc.vector.tensor_copy(o_sb[:, 0:NSPLIT], ps0[:])
                            nc.scalar.dma_start(
                                out=out_d[row : row + P, 0:NSPLIT],
                                in_=o_sb[:, 0:NSPLIT],
                            )
                            for kt in range(KT):
                                nc.tensor.matmul(
                                    ps1[:], lhs[kt], rhs[kt][:, NSPLIT:OUT_C],
                                    start=(kt == 0), stop=(kt == KT - 1),
                                )
                            nc.vector.tensor_copy(o_sb[:, NSPLIT:OUT_C], ps1[:])
                            nc.scalar.dma_start(
                                out=out_d[row : row + P, NSPLIT:OUT_C],
                                in_=o_sb[:, NSPLIT:OUT_C],
                            )
    legalize_waits(nc)
    return nc


def _prepare(input, weight, expert_offsets):
    offs = np.asarray(expert_offsets).astype(np.int64)
    sizes, padded_sizes, Tp = _pad_segments(offs)
    x = np.asarray(input, dtype=np.float32)
    w = np.asarray(weight, dtype=np.float32)

    if Tp == T and all(s == p for s, p in zip(sizes, padded_sizes)):
        xT = np.ascontiguousarray(x.T).astype(ml_dtypes.bfloat16)
    else:
        xp = np.zeros((Tp, IN), dtype=np.float32)
        base = 0
        for e in range(E):
            s, sz = int(offs[e]), sizes[e]
            xp[base : base + sz] = x[s : s + sz]
            base += padded_sizes[e]
        xT = np.ascontiguousarray(xp.T).astype(ml_dtypes.bfloat16)

    wb = w.astype(ml_dtypes.bfloat16)
    in_maps = []
    for c in range(NCORES):
        wTc = np.ascontiguousarray(
            wb[:, c * OUT_C : (c + 1) * OUT_C, :].transpose(0, 2, 1)
        )
        in_maps.append({"xT": xT, "wT": wTc})
    return sizes, padded_sizes, Tp, in_maps


def _gather(results, sizes, padded_sizes):
    full = np.concatenate(
        [r["out"].astype(np.float32) for r in results], axis=1
    )
    if sum(sizes) == full.shape[0]:
        return full
    out = np.empty((sum(sizes), OUT), dtype=np.float32)
    base_p = base = 0
    for e in range(E):
        out[base : base + sizes[e]] = full[base_p : base_p + sizes[e]]
        base += sizes[e]
        base_p += padded_sizes[e]
    return out


def run(input, weight, expert_offsets, trace=False):
    import concourse.mybir as mybir
    from concourse.bass_utils import run_bass_kernel_spmd

    sizes, padded_sizes, Tp, in_maps = _prepare(input, weight, expert_offsets)
    nc = _build_program(padded_sizes, mybir.dt.bfloat16, mybir.dt.bfloat16)
    core_ids = list(range(NCORES))
    res = run_bass_kernel_spmd(nc, in_maps, core_ids, trace=trace)
    out = _gather(res.results, sizes, padded_sizes)
    return out, res


def kernel(input, weight, expert_offsets):
    out, _ = run(input, weight, expert_offsets)
    return out


# --- embedded helper (kernel.py must be self-contained) ---------------------
import sys as _sys
import types as _types

_wl_src = '''
import concourse.mybir as mybir


def legalize_waits(nc, maxw: int = 1) -> int:
    """Walrus accepts a limited number of sync-wait commands per instruction;
    split extras onto preceding same-engine NOPs (one wait each)."""
    split = 0
    for f in nc.m.functions:
        for blk in f.blocks:
            new_instructions = []
            for inst in blk.instructions:
                si = inst.sync_info
                waits = list(si.on_wait) if si and si.on_wait else []
                if len(waits) > maxw:
                    keep = waits[-maxw:]
                    extra = waits[:-maxw]
                    for w in extra:
                        nop = mybir.InstNoOp(
                            name=nc.get_next_instruction_name(),
                            sync_info=mybir.SyncInfo(on_wait=[w], on_update=[]),
                            bass_nofuse=True,
                            engine=inst.engine,
                        )
                        new_instructions.append(nop)
                        split += 1
                    inst.sync_info = mybir.SyncInfo(
                        on_wait=keep,
                        on_update=list(si.on_update) if si.on_update else [],
                    )
                new_instructions.append(inst)
            blk.instructions = new_instructions
    return split
'''

_wl_mod = _types.ModuleType("wait_legalize_embed")
exec(_wl_src, _wl_mod.__dict__)
_sys.modules["wait_legalize_embed"] = _wl_mod


# revision 5
# speedup vs baseline: 1.0025x; 1.0025x over previous
"""Grouped GEMM (MoE expert layers) on 8 Trainium2 NeuronCores.

Sharding: tensor-parallel over out_features — core c owns a contiguous
704-wide slice of OUT and sees all 16384 tokens, so the expert
segmentation enters only as identical trace-time loop bounds on every
core (one SPMD program). Operands are cast to bf16 on the host (rel err
~2.6e-3 vs the 2e-2 gate); accumulation is fp32 in PSUM; output is
written bf16 and widened to f32 on the host.

Per core: the PE streams 4096 back-to-back matmuls (N=352 psum-bank
halves, K-contiguous per 128-token tile) = ~614 us busy at 2.4 GHz,
within ~3% of this sharding's bf16 roofline. Everything else is shaped
to keep the PE dense:
  - w transfers ride the scalar HWDGE ring, x the sync ring, outputs
    the scalar ring; each ring handles one DMA end-to-end, so the
    streams never queue behind each other.
  - w/x tiles are split into kt-group sub-tiles with per-tile
    dependencies, so the first matmul waits only for its own slice.
  - ~32 warmup matmuls on a zeroed tile run during the initial DMA fill
    to hold the PE HAM clock-gate at 2.4 GHz before real work starts.
  - the first 4-tile chunk runs kt-group-major across all 8 psum banks,
    matching compute order to DMA arrival order during pipe-fill.
  - the last tile finishes psum half 0 early so its cast + output DMA
    overlap half 1's matmuls.
"""
import os
import numpy as np
import ml_dtypes

E, IN, OUT, T, NCORES = 8, 2048, 5632, 16384, 8
OUT_C = OUT // NCORES          # 704 out-features per core
P = 128                        # partitions
KT = IN // P                   # 16 k-tiles of 128
NSPLIT = 352                   # psum bank-sized halves of OUT_C
WARMUP_MM = int(os.environ.get("V4_WARMUP", "32"))


def _pad_segments(offsets):
    sizes = np.diff(offsets).astype(int)
    padded = [(-(-s // P)) * P for s in sizes]
    return list(sizes), padded, int(sum(padded))


def _build_program(padded_sizes, dt_in, dt_out):
    import concourse.bass as bass
    import concourse.mybir as mybir
    from concourse.tile import TileContext
    from wait_legalize_embed import legalize_waits

    Tp = sum(padded_sizes)
    nc = bass.Bass()
    xT_d = nc.dram_tensor("xT", [IN, Tp], dt_in, kind="ExternalInput")
    wT_d = nc.dram_tensor("wT", [E, IN, OUT_C], dt_in, kind="ExternalInput")
    out_d = nc.dram_tensor("out", [Tp, OUT_C], dt_out, kind="ExternalOutput")

    xT_r = xT_d.rearrange("(kt p) t -> p kt t", p=P)

    # expert-order list of (global_tile0, ntiles) segments
    segs = []
    tb = 0
    for e in range(E):
        nt = padded_sizes[e] // P
        if nt:
            segs.append((e, tb, nt))
        tb += nt
    last_seg = len(segs) - 1

    with TileContext(nc) as tc:
        with tc.tile_pool(name="wpool", bufs=2) as wpool, \
             tc.tile_pool(name="xpool", bufs=2) as xpool, \
             tc.tile_pool(name="zpool", bufs=1) as zpool, \
             tc.tile_pool(name="opool", bufs=6) as opool, \
             tc.tile_pool(name="ppool", bufs=8, space="PSUM") as ppool:
            # ---- PE warmup: run garbage matmuls while the first DMAs land
            # (the psum comes from the shared "ps" ring, so it is recycled
            # once the first chunk needs all 8 banks)
            if WARMUP_MM:
                z_sb = zpool.tile([P, P], dt_in, tag="z", name="zsb")
                nc.vector.memset(z_sb[:], 0.0)
                psw = ppool.tile([P, NSPLIT], mybir.dt.float32, tag="ps",
                                 name="psw")
                for _ in range(WARMUP_MM):
                    nc.tensor.matmul(psw[:, 0:P], z_sb[:], z_sb[:],
                                     start=True, stop=True)

            for si, (e, tile0, ntiles) in enumerate(segs):
                first = si == 0
                # w DMA: 2-kt groups for the first expert, 4-kt after
                kgn_w = 2 if first else 4
                ngw = KT // kgn_w
                w_r = wT_d[e].rearrange("(kt p) n -> p kt n", p=P)
                wtag = "v" if first else "w"
                w_sb = [wpool.tile([P, kgn_w, OUT_C], dt_in,
                                   tag=f"{wtag}{g}", name=f"wsb{g}",
                                   bufs=1 if first else 2)
                        for g in range(ngw)]
                for g in range(ngw):
                    nc.scalar.dma_start(
                        out=w_sb[g][:],
                        in_=w_r[:, g * kgn_w : (g + 1) * kgn_w, :],
                    )

                # token chunking: first chunk of first expert is 4 tiles
                chunks = []
                t = 0
                while t < ntiles:
                    step = 4 if (first and t == 0) else 8
                    cur = min(step, ntiles - t)
                    chunks.append((t, cur))
                    t += cur
                for ci, (tt0, cur) in enumerate(chunks):
                    t0 = (tile0 + tt0) * P
                    small = first and ci == 0
                    kgn_x = 2 if small else 4
                    ngx = KT // kgn_x
                    xtag = "y" if small else "x"
                    xw = 4 * P if small else 8 * P
                    x_sb = [xpool.tile([P, kgn_x, xw], dt_in,
                                       tag=f"{xtag}{g}", name=f"xsb{g}",
                                       bufs=1 if small else 2)
                            for g in range(ngx)]
                    for g in range(ngx):
                        nc.sync.dma_start(
                            out=x_sb[g][:, :, : cur * P],
                            in_=xT_r[:, g * kgn_x : (g + 1) * kgn_x,
                                     t0 : t0 + cur * P],
                        )
                    if small:
                        # kt-group-major over the whole first chunk: each
                        # newly-landed 2-kt slice immediately feeds 4x4
                        # matmuls, so the PE never outruns the DMA cadence.
                        # Uses 2*cur psum tiles (= all 8 banks at cur=4).
                        pss = [
                            (ppool.tile([P, NSPLIT], mybir.dt.float32,
                                        tag="ps", name=f"psA{j}"),
                             ppool.tile([P, NSPLIT], mybir.dt.float32,
                                        tag="ps", name=f"psB{j}"))
                            for j in range(cur)
                        ]
                        for kt in range(KT):
                            gx, kx = kt // kgn_x, kt % kgn_x
                            gw, kw = kt // kgn_w, kt % kgn_w
                            for j in range(cur):
                                lhsT = x_sb[gx][:, kx, j * P : (j + 1) * P]
                                nc.tensor.matmul(
                                    pss[j][0][:], lhsT,
                                    w_sb[gw][:, kw, 0:NSPLIT],
                                    start=(kt == 0), stop=(kt == KT - 1),
                                )
                                nc.tensor.matmul(
                                    pss[j][1][:], lhsT,
                                    w_sb[gw][:, kw, NSPLIT:OUT_C],
                                    start=(kt == 0), stop=(kt == KT - 1),
                                )
                        for j in range(cur):
                            o_sb = opool.tile([P, OUT_C], dt_out, tag="o")
                            nc.vector.tensor_copy(o_sb[:, 0:NSPLIT], pss[j][0][:])
                            nc.vector.tensor_copy(o_sb[:, NSPLIT:OUT_C], pss[j][1][:])
                            row = t0 + j * P
                            nc.scalar.dma_start(
                                out=out_d[row : row + P, :], in_=o_sb[:]
                            )
                        continue
                    for j in range(cur):
                        ps0 = ppool.tile([P, NSPLIT], mybir.dt.float32, tag="ps")
                        ps1 = ppool.tile([P, NSPLIT], mybir.dt.float32, tag="ps")
                        lhs = []
                        for kt in range(KT):
                            gx, kx = kt // kgn_x, kt % kgn_x
                            lhs.append(x_sb[gx][:, kx, j * P : (j + 1) * P])
                        rhs = []
                        for kt in range(KT):
                            gw, kw = kt // kgn_w, kt % kgn_w
                            rhs.append(w_sb[gw][:, kw, :])
                        tail = si == last_seg and ci == len(chunks) - 1 \
                            and j == cur - 1
                        o_sb = opool.tile([P, OUT_C], dt_out, tag="o")
                        row = t0 + j * P
                        if not tail:
                            for kt in range(KT):
                                nc.tensor.matmul(
                                    ps0[:], lhs[kt], rhs[kt][:, 0:NSPLIT],
                                    start=(kt == 0), stop=(kt == KT - 1),
                                )
                                nc.tensor.matmul(
                                    ps1[:], lhs[kt], rhs[kt][:, NSPLIT:OUT_C],
                                    start=(kt == 0), stop=(kt == KT - 1),
                                )
                            nc.vector.tensor_copy(o_sb[:, 0:NSPLIT], ps0[:])
                            nc.vector.tensor_copy(o_sb[:, NSPLIT:OUT_C], ps1[:])
                            nc.scalar.dma_start(
                                out=out_d[row : row + P, :], in_=o_sb[:]
                            )
                        else:
                            # last tile: finish ps0 early so its cast +
                            # output DMA overlap ps1's matmuls
                            for kt in range(KT):
                                nc.tensor.matmul(
                                    ps0[:], lhs[kt], rhs[kt][:, 0:NSPLIT],
                                    start=(kt == 0), stop=(kt == KT - 1),
                                )
                            nc.vector.tensor_copy(o_sb[:, 0:NSPLIT], ps0[:])
                            nc.scalar.dma_start(
                                out=out_d[row : row + P, 0:NSPLIT],
                                in_=o_sb[:, 0:NSPLIT],
                            )
                            for kt in range(KT):
                                nc.tensor.matmul(
                                    ps1[:], lhs[kt], rhs[kt][:, NSPLIT:OUT_C],
                                    start=(kt == 0), stop=(kt == KT - 1),
                                )
                            nc.vector.tensor_copy(o_sb[:, NSPLIT:OUT_C], ps1[:])
                            nc.scalar.dma_start(
                                out=out_d[row : row + P, NSPLIT:OUT_C],
                                in_=o_sb[:, NSPLIT:OUT_C],
                            )
    legalize_waits(nc)
    return nc


def _prepare(input, weight, expert_offsets):
    offs = np.asarray(expert_offsets).astype(np.int64)
    sizes, padded_sizes, Tp = _pad_segments(offs)
    x = np.asarray(input, dtype=np.float32)
    w = np.asarray(weight, dtype=np.float32)

    if Tp == T and all(s == p for s, p in zip(sizes, padded_sizes)):
        xT = np.ascontiguousarray(x.T).astype(ml_dtypes.bfloat16)
    else:
        xp = np.zeros((Tp, IN), dtype=np.float32)
        base = 0
        for e in range(E):
            s, sz = int(offs[e]), sizes[e]
            xp[base : base + sz] = x[s : s + sz]
            base += padded_sizes[e]
        xT = np.ascontiguousarray(xp.T).astype(ml_dtypes.bfloat16)

    wb = w.astype(ml_dtypes.bfloat16)
    in_maps = []
    for c in range(NCORES):
        wTc = np.ascontiguousarray(
            wb[:, c * OUT_C : (c + 1) * OUT_C, :].transpose(0, 2, 1)
        )
        in_maps.append({"xT": xT, "wT": wTc})
    return sizes, padded_sizes, Tp, in_maps


def _gather(results, sizes, padded_sizes):
    full = np.concatenate(
        [r["out"].astype(np.float32) for r in results], axis=1
    )
    if sum(sizes) == full.shape[0]:
        return full
    out = np.empty((sum(sizes), OUT), dtype=np.float32)
    base_p = base = 0
    for e in range(E):
        out[base : base + sizes[e]] = full[base_p : base_p + sizes[e]]
        base += sizes[e]
        base_p += padded_sizes[e]
    return out


def run(input, weight, expert_offsets, trace=False):
    import concourse.mybir as mybir
    from concourse.bass_utils import run_bass_kernel_spmd

    sizes, padded_sizes, Tp, in_maps = _prepare(input, weight, expert_offsets)
    nc = _build_program(padded_sizes, mybir.dt.bfloat16, mybir.dt.bfloat16)
    core_ids = list(range(NCORES))
    res = run_bass_kernel_spmd(nc, in_maps, core_ids, trace=trace)
    out = _gather(res.results, sizes, padded_sizes)
    return out, res


def kernel(input, weight, expert_offsets):
    out, _ = run(input, weight, expert_offsets)
    return out


# --- embedded helper (kernel.py must be self-contained) ---------------------
import sys as _sys
import types as _types

_wl_src = '''
import concourse.mybir as mybir


def legalize_waits(nc, maxw: int = 1) -> int:
    """Walrus accepts a limited number of sync-wait commands per instruction;
    split extras onto preceding same-engine NOPs (one wait each)."""
    split = 0
    for f in nc.m.functions:
        for blk in f.blocks:
            new_instructions = []
            for inst in blk.instructions:
                si = inst.sync_info
                waits = list(si.on_wait) if si and si.on_wait else []
                if len(waits) > maxw:
                    keep = waits[-maxw:]
                    extra = waits[:-maxw]
                    for w in extra:
                        nop = mybir.InstNoOp(
                            name=nc.get_next_instruction_name(),
                            sync_info=mybir.SyncInfo(on_wait=[w], on_update=[]),
                            bass_nofuse=True,
                            engine=inst.engine,
                        )
                        new_instructions.append(nop)
                        split += 1
                    inst.sync_info = mybir.SyncInfo(
                        on_wait=keep,
                        on_update=list(si.on_update) if si.on_update else [],
                    )
                new_instructions.append(inst)
            blk.instructions = new_instructions
    return split
'''

_wl_mod = _types.ModuleType("wait_legalize_embed")
exec(_wl_src, _wl_mod.__dict__)
_sys.modules["wait_legalize_embed"] = _wl_mod
